# revision 33
# baseline (speedup 1.0000x reference)
"""Distributed causal multi-head attention for TRN2 (8 NeuronCores).

Problem: B=2, T=2048, D=1024, H=16 heads (head_dim 64), causal MHA:
  q,k,v = x@W{q,k,v}+b, q *= dh**-0.5, o = softmax(mask(q k^T)) v, out = o@Wp + bp

Sharding: 8-way tensor parallel over heads.  Core r handles BOTH batches,
heads {2r, 2r+1}, and output columns 128r..128(r+1).  This makes the
AllGather a single uniform 8-core collective (the 4-core-group path
measured ~50us/call vs ~7us for 8-core) with every gathered byte used by
every core.  Per core:
  - QKV projections in fp16 on TensorE (q/k produced transposed [hd, t],
    v produced natural [t, hd] with an appended ones-column)
  - scores computed transposed [keys, q] (K=64 contraction, two heads
    packed into the 128x128 PE array via row tiling, both writing halves
    of one 2-bank PSUM tile); ONE fused exp per key tile on ScalarE
    (the pipeline pacer); causal handled by key-tile skipping + a
    post-exp 0/1 mask multiply on the diagonal blocks
  - AV uses exp-weights as the stationary operand -> o natural [q, hd]
    with per-partition row sums for free (ones column of v); normalize
    with a per-partition reciprocal; AV interleaves with scores at lag 1
    so the PE never waits long on ScalarE
  - o is PE-transposed locally (cheap) so the AllGather carries oT and
    the output projection needs no DMA transposes
  - output projection computes a 128-column slice per core, pipelined
    one chunk behind the AllGather.
Host side only shards/converts inputs, concatenates outputs, and adds
the bias terms that are mathematically output-constant (bv@Wp + bp; bk
cancels in softmax; bq is applied on device).
"""

import os
import numpy as np

B, T, D, H = 2, 2048, 1024, 16
DH = 64
NCORES = 8
HPC = H // NCORES      # heads per core = 2
CD = HPC * DH          # per-core head-dim / out columns = 128
P = 128
NCH = 4                # T chunks for the AllGather pipeline
CHUNK = T // NCH       # 512
KT = T // P            # 16 key tiles
KD = D // P            # 8 contraction tiles for the projections

_CACHE = {}

# Results of the last device run (for test harnesses): BassKernelResults
LAST_RESULT = None


def _build_nc():
    import concourse.bass as bass
    import concourse.mybir as mybir
    import concourse.tile as tile
    from concourse import bacc
    from contextlib import ExitStack

    fp = mybir.dt.float16
    f32 = mybir.dt.float32
    AF = mybir.ActivationFunctionType

    nc = bacc.Bacc("TRN2", target_bir_lowering=False, debug=False,
                   num_devices=NCORES)

    xT = nc.dram_tensor("xT", [D, B, T], fp, kind="ExternalInput").ap()
    wq = nc.dram_tensor("wq", [D, CD], fp, kind="ExternalInput").ap()
    wk = nc.dram_tensor("wk", [D, CD], fp, kind="ExternalInput").ap()
    wv = nc.dram_tensor("wv", [D, CD], fp, kind="ExternalInput").ap()
    wp = nc.dram_tensor("wp", [D, CD], fp, kind="ExternalInput").ap()
    bqp = nc.dram_tensor("bqp", [P, 1], f32, kind="ExternalInput").ap()
    maskf = nc.dram_tensor("maskf", [P, P], fp, kind="ExternalInput").ap()
    ident = nc.dram_tensor("ident", [P, P], fp, kind="ExternalInput").ap()
    out = nc.dram_tensor("out", [B, T, CD], f32, kind="ExternalOutput").ap()

    obounce = nc.dram_tensor("obounce", [NCH, B, CD, CHUNK], fp).ap()
    gath = nc.dram_tensor("gath", [NCH, NCORES, B, CD, CHUNK], fp).ap()
    warm_in = nc.dram_tensor("warm_in", [P], fp).ap()
    warm_out = nc.dram_tensor("warm_out", [NCORES, P], fp).ap()

    RG = [[0, 1, 2, 3, 4, 5, 6, 7]]

    with tile.TileContext(nc, num_cores=NCORES) as tc, ExitStack() as ctx:
        const = ctx.enter_context(tc.tile_pool(name="const", bufs=1))
        work = ctx.enter_context(tc.tile_pool(name="work", bufs=3))
        expp = ctx.enter_context(tc.tile_pool(name="expp", bufs=18))
        otkp = ctx.enter_context(tc.tile_pool(name="otkp", bufs=18))
        osbp = ctx.enter_context(tc.tile_pool(name="osbp", bufs=8))
        psum = ctx.enter_context(tc.tile_pool(name="psum", bufs=2,
                                              space="PSUM"))

        # ---- persistent SBUF ----
        xT_sb = const.tile([P, KD, B, T], fp)        # 64 KB/p
        wq_sb = const.tile([P, KD, CD], fp)
        wk_sb = const.tile([P, KD, CD], fp)
        wv_sb = const.tile([P, KD, CD], fp)
        wp_sb = const.tile([P, KD, CD], fp)
        bq_sb = const.tile([P, 1], f32)
        mask_sb = const.tile([P, P], fp)             # 0/1 lower triangle
        ident_sb = const.tile([P, P], fp)
        qT_sb = const.tile([P, B, T], fp)            # 2 heads stacked
        kT_sb = const.tile([P, B, T], fp)
        v_sb = const.tile([P, KT, B, HPC, DH + 1], fp)

        # weights first so the first projection matmuls start immediately;
        # x streams in per chunk behind them
        nc.sync.dma_start(wq_sb[:], wq.rearrange("(k p) c -> p k c", p=P))
        nc.sync.dma_start(wk_sb[:], wk.rearrange("(k p) c -> p k c", p=P))
        nc.sync.dma_start(wv_sb[:], wv.rearrange("(k p) c -> p k c", p=P))
        nc.sync.dma_start(wp_sb[:], wp.rearrange("(k p) c -> p k c", p=P))
        nc.sync.dma_start(bq_sb[:], bqp)
        nc.sync.dma_start(mask_sb[:], maskf)
        nc.sync.dma_start(ident_sb[:], ident)
        nc.vector.memset(v_sb[:, :, :, :, DH:DH + 1], 1.0)
        xT_r = xT.rearrange("(k p) b t -> p k b t", p=P)
        for t4 in range(NCH):
            # split the 8MB load so chunk-0 compute starts immediately
            for b in range(B):
                nc.sync.dma_start(
                    xT_sb[:, :, b, t4 * 512:(t4 + 1) * 512],
                    xT_r[:, :, b, t4 * 512:(t4 + 1) * 512])
        # tiny warmup collective: absorbs the first-collective latency
        # anomaly while the input DMAs stream
        nc.gpsimd.collective_compute(
            "AllGather", bass.mybir.AluOpType.bypass,
            replica_groups=RG, ins=[warm_in], outs=[warm_out])

        def qkv_chunk(t4):
            """Project q,k (transposed) and v (natural) for T-chunk t4."""
            for b in range(B):
                psqk = psum.tile([P, 1024], f32, tag="big",
                                 name=f"psqk_{t4}_{b}")
                for k in range(KD):
                    nc.tensor.matmul(
                        psqk[:, 0:512], wq_sb[:, k, :],
                        xT_sb[:, k, b, t4 * 512:(t4 + 1) * 512],
                        start=(k == 0), stop=(k == KD - 1))
                for k in range(KD):
                    nc.tensor.matmul(
                        psqk[:, 512:1024], wk_sb[:, k, :],
                        xT_sb[:, k, b, t4 * 512:(t4 + 1) * 512],
                        start=(k == 0), stop=(k == KD - 1))
                nc.vector.tensor_scalar_add(
                    qT_sb[:, b, t4 * 512:(t4 + 1) * 512], psqk[:, 0:512],
                    bq_sb[:, 0:1])
                nc.vector.tensor_copy(
                    kT_sb[:, b, t4 * 512:(t4 + 1) * 512], psqk[:, 512:1024])
                for tt in range(4 * t4, 4 * t4 + 4):
                    psv = psum.tile([P, 256], f32, tag="pv",
                                    name=f"psv_{tt}_{b}")
                    for k in range(KD):
                        nc.tensor.matmul(
                            psv[:, :CD], xT_sb[:, k, b, tt * P:(tt + 1) * P],
                            wv_sb[:, k, :], start=(k == 0),
                            stop=(k == KD - 1))
                    nc.vector.tensor_copy(
                        out=v_sb[:, tt, b, :, 0:DH],
                        in_=psv[:, :CD].rearrange("p (h d) -> p h d", h=HPC))

        def attention_batch(c, b):
            """Causal attention for q-chunk c, batch b (2 heads packed).

            scores for both heads go into one [128,1024] PSUM tile
            (row-packed K=64 matmuls -> halves), one fused exp per key
            tile, AV interleaved with lag 1.  AV accumulates all four
            q-subtiles of each head in one PSUM bank (4 interleaved
            accumulation groups as column ranges)."""
            nkt = 4 * (c + 1)
            exp_tiles = {}
            pso = {}
            for hh in range(2):
                pso[hh] = psum.tile([P, 4, DH + 1], f32, tag="o",
                                    name=f"pso_{c}_{b}_{hh}")

            def do_scores(k):
                ps_s = psum.tile([P, 1024], f32, tag="big",
                                 name=f"ps_{c}_{b}_{k}")
                for hh in range(2):
                    lo, hi = hh * DH, (hh + 1) * DH
                    nc.tensor.matmul(
                        ps_s[:, hh * 512:(hh + 1) * 512],
                        kT_sb[lo:hi, b, k * P:(k + 1) * P],
                        qT_sb[lo:hi, b, c * 512:(c + 1) * 512],
                        start=True, stop=True)
                e = expp.tile([P, 1024], fp, tag="expT",
                              name=f"expT_{c}_{b}_{k}")
                nc.scalar.activation(e[:], ps_s[:], AF.Exp)
                j = k - 4 * c
                if j >= 0:
                    blks = e[:].rearrange("p (hh q) -> p hh q", hh=2)[
                        :, :, j * P:(j + 1) * P]
                    nc.vector.tensor_mul(
                        blks, blks,
                        mask_sb[:, None, :].to_broadcast([P, 2, P]))
                exp_tiles[k] = e

            def do_av(k):
                # pso[hh] holds 4 interleaved accumulation groups in one
                # PSUM bank; only the first write of the bank (k==0,s==0)
                # may set start (bank-wide has_written clear).
                for hh in range(2):
                    h = 2 * b + hh  # local index only
                    for s in range(4):
                        if k <= 4 * c + s:
                            nc.tensor.matmul(
                                pso[hh][:, s, :],
                                exp_tiles[k][:, hh * 512 + s * P:
                                             hh * 512 + (s + 1) * P],
                                v_sb[:, k, b, hh, :],
                                start=(k == 0 and s == 0),
                                stop=(k == 4 * c + s),
                                skip_group_check=True)

            for k in range(nkt + 1):
                if k < nkt:
                    do_scores(k)
                if k > 0:
                    do_av(k - 1)
            return pso

        def proj_loads(c):
            """Plain DMA loads of the gathered (already transposed) heads."""
            oTk = {}
            for b2 in range(B):
                for k in range(KD):
                    t_ = otkp.tile([P, CHUNK], fp, tag="oTk",
                                   name=f"oTk_{c}_{b2}_{k}")
                    nc.sync.dma_start(t_[:], gath[c, k, b2])
                    oTk[(b2, k)] = t_
            return oTk

        def proj_chunk(c, oTk):
            """Output projection for T-chunk c (both batches in one
            2-bank PSUM tile, 4 q-subtile groups per bank)."""
            psp = psum.tile([P, B, 4, CD], f32, tag="big",
                            name=f"psp_{c}")
            for b2 in range(B):
                for k in range(KD):
                    for s in range(4):
                        nc.tensor.matmul(
                            psp[:, b2, s, :], oTk[(b2, k)][:, s * P:(s + 1) * P],
                            wp_sb[:, k, :],
                            start=(k == 0 and s == 0), stop=(k == KD - 1),
                            skip_group_check=True)
            outsb = work.tile([P, B, 4, CD], f32, tag="outsb",
                              name=f"outsb_{c}")
            nc.vector.tensor_copy(outsb[:], psp[:])
            for b2 in range(B):
                nc.sync.dma_start(
                    out[b2, c * 512:(c + 1) * 512, :].rearrange(
                        "(s p) col -> p s col", p=P),
                    outsb[:, b2])

        # pipeline: attention(c) paces ScalarE; per-chunk AllGathers fly
        # while later chunks compute; qkv(c+1) and proj(c-1) fill the PE
        # while ScalarE drains exps and AG(c) is in flight.
        qkv_chunk(0)
        for c in range(NCH):
            for b in range(B):
                pso = attention_batch(c, b)
                osb = osbp.tile([P, 4, CD], fp, tag="osb",
                                name=f"osb_{c}_{b}")
                for hh in range(2):
                    for s in range(4):
                        rec = work.tile([P, 1], f32, tag="rec",
                                        name=f"rec_{c}_{b}_{hh}_{s}")
                        nc.vector.reciprocal(rec[:],
                                             pso[hh][:, s, DH:DH + 1])
                        nc.vector.tensor_scalar_mul(
                            osb[:, s, hh * DH:(hh + 1) * DH],
                            pso[hh][:, s, 0:DH], rec[:])
                # local PE transpose: obounce carries oT so the
                # projection needs no DMA transposes
                obT = work.tile([P, 4, P], fp, tag="obT",
                                name=f"obT_{c}_{b}")
                for s in range(4):
                    trp = psum.tile([P, P], fp, tag="pv",
                                    name=f"trp_{c}_{b}_{s}")
                    nc.tensor.transpose(trp[:], osb[:, s, :], ident_sb[:])
                    nc.vector.tensor_copy(obT[:, s, :], trp[:])
                nc.sync.dma_start(
                    obounce[c, b].rearrange("p (s t) -> p s t", t=P),
                    obT[:])
            nc.gpsimd.collective_compute(
                "AllGather", bass.mybir.AluOpType.bypass,
                replica_groups=RG,
                ins=[obounce[c]],
                outs=[gath[c]])
            if c + 1 < NCH:
                qkv_chunk(c + 1)
            if c >= 1:
                # AG(c-1) completed during attention(c); loads + matmuls
                # here so they never head-of-line block the PE or sync
                # queue for attention work
                proj_chunk(c - 1, proj_loads(c - 1))
        proj_chunk(NCH - 1, proj_loads(NCH - 1))

    nc.finalize()
    return nc


def _get_nc():
    if "nc" not in _CACHE:
        _CACHE["nc"] = _build_nc()
    return _CACHE["nc"]


def kernel(x, Wq, bq, Wk, bk, Wv, bv, Wp, bp):
    global LAST_RESULT
    from concourse.bass_utils import run_bass_kernel_spmd

    x = np.asarray(x, dtype=np.float32)
    Wq = np.asarray(Wq, dtype=np.float32)
    Wk = np.asarray(Wk, dtype=np.float32)
    Wv = np.asarray(Wv, dtype=np.float32)
    Wp = np.asarray(Wp, dtype=np.float32)
    bq = np.asarray(bq, dtype=np.float32)
    bv = np.asarray(bv, dtype=np.float32)
    bp = np.asarray(bp, dtype=np.float32)

    s = DH ** -0.5
    maskf = np.where(
        np.arange(P)[:, None] <= np.arange(P)[None, :], 1.0, 0.0
    ).astype(np.float16)
    ident = np.eye(P, dtype=np.float16)
    xTg = np.ascontiguousarray(np.stack([x[0].T, x[1].T], axis=1)
                               ).astype(np.float16)

    in_maps = []
    for r in range(NCORES):
        cols = slice(r * CD, (r + 1) * CD)
        in_maps.append({
            "xT": xTg,
            "wq": (Wq[:, cols] * s).astype(np.float16),
            "wk": np.ascontiguousarray(Wk[:, cols]).astype(np.float16),
            "wv": np.ascontiguousarray(Wv[:, cols]).astype(np.float16),
            "wp": np.ascontiguousarray(Wp[:, cols]).astype(np.float16),
            "bqp": np.ascontiguousarray((bq[cols] * s).reshape(P, 1)),
            "maskf": maskf,
            "ident": ident,
        })

    nc = _get_nc()
    res = run_bass_kernel_spmd(
        nc, in_maps, core_ids=list(range(NCORES)),
        trace=bool(int(os.environ.get("KERNEL_TRACE", "0"))))
    LAST_RESULT = res

    out = np.empty((B, T, D), dtype=np.float32)
    for r in range(NCORES):
        out[:, :, r * CD:(r + 1) * CD] = res.results[r]["out"]
    # bias terms that are constant w.r.t. the data path:
    #   v-bias passes through softmax rows (sum=1) -> + bv@Wp; plus bp.
    #   (bk shifts every logit in a row equally -> cancels in softmax.)
    out += (bv @ Wp + bp)[None, None, :]
    return out


# revision 35
# speedup vs baseline: 1.4320x; 1.4320x over previous
"""Distributed causal multi-head attention for TRN2 (8 NeuronCores).

Problem: B=2, T=2048, D=1024, H=16 heads (head_dim 64), causal MHA:
  q,k,v = x@W{q,k,v}+b, q *= dh**-0.5, o = softmax(mask(q k^T)) v, out = o@Wp + bp

Sharding: 8-way tensor parallel over heads.  Core r handles BOTH batches,
heads {2r, 2r+1}, and output columns 128r..128(r+1).  This makes the
AllGather a single uniform 8-core collective (the 4-core-group path
measured ~50us/call vs ~7us for 8-core) with every gathered byte used by
every core.  Per core:
  - QKV projections in fp16 on TensorE (q/k produced transposed [hd, t],
    v produced natural [t, hd] with an appended ones-column)
  - scores computed transposed [keys, q] (K=64 contraction, two heads
    packed into the 128x128 PE array via row tiling, both writing halves
    of one 2-bank PSUM tile); ONE fused exp per key tile on ScalarE
    (the pipeline pacer); causal handled by key-tile skipping + a
    post-exp 0/1 mask multiply on the diagonal blocks
  - AV uses exp-weights as the stationary operand -> o natural [q, hd]
    with per-partition row sums for free (ones column of v); normalize
    with a per-partition reciprocal; AV interleaves with scores at lag 1
    so the PE never waits long on ScalarE
  - o is PE-transposed locally (cheap) so the AllGather carries oT and
    the output projection needs no DMA transposes
  - output projection computes a 128-column slice per core, pipelined
    one chunk behind the AllGather.
Host side only shards/converts inputs, concatenates outputs, and adds
the bias terms that are mathematically output-constant (bv@Wp + bp; bk
cancels in softmax; bq is applied on device).
"""

import os
import numpy as np

B, T, D, H = 2, 2048, 1024, 16
DH = 64
NCORES = 8
HPC = H // NCORES      # heads per core = 2
CD = HPC * DH          # per-core head-dim / out columns = 128
P = 128
NCH = 4                # T chunks for the AllGather pipeline
CHUNK = T // NCH       # 512
KT = T // P            # 16 key tiles
KD = D // P            # 8 contraction tiles for the projections

_CACHE = {}

# Results of the last device run (for test harnesses): BassKernelResults
LAST_RESULT = None


def _build_nc():
    import concourse.bass as bass
    import concourse.mybir as mybir
    import concourse.tile as tile
    from concourse import bacc
    from contextlib import ExitStack

    fp = mybir.dt.float16
    f32 = mybir.dt.float32
    AF = mybir.ActivationFunctionType

    nc = bacc.Bacc("TRN2", target_bir_lowering=False, debug=False,
                   num_devices=NCORES)

    xT = nc.dram_tensor("xT", [D, B, T], fp, kind="ExternalInput").ap()
    wq = nc.dram_tensor("wq", [D, CD], fp, kind="ExternalInput").ap()
    wk = nc.dram_tensor("wk", [D, CD], fp, kind="ExternalInput").ap()
    wv = nc.dram_tensor("wv", [D, CD], fp, kind="ExternalInput").ap()
    wp = nc.dram_tensor("wp", [D, CD], fp, kind="ExternalInput").ap()
    bqp = nc.dram_tensor("bqp", [P, 1], f32, kind="ExternalInput").ap()
    maskf = nc.dram_tensor("maskf", [P, P], fp, kind="ExternalInput").ap()
    ident = nc.dram_tensor("ident", [P, P], fp, kind="ExternalInput").ap()
    out = nc.dram_tensor("out", [B, T, CD], f32, kind="ExternalOutput").ap()

    obounce = nc.dram_tensor("obounce", [NCH, B, CD, CHUNK], fp).ap()
    gath = nc.dram_tensor("gath", [NCH, NCORES, B, CD, CHUNK], fp).ap()
    warm_in = nc.dram_tensor("warm_in", [P], fp).ap()
    warm_out = nc.dram_tensor("warm_out", [NCORES, P], fp).ap()

    RG = [[0, 1, 2, 3, 4, 5, 6, 7]]

    with tile.TileContext(nc, num_cores=NCORES) as tc, ExitStack() as ctx:
        const = ctx.enter_context(tc.tile_pool(name="const", bufs=1))
        work = ctx.enter_context(tc.tile_pool(name="work", bufs=3))
        expp = ctx.enter_context(tc.tile_pool(name="expp", bufs=18))
        otkp = ctx.enter_context(tc.tile_pool(name="otkp", bufs=18))
        osbp = ctx.enter_context(tc.tile_pool(name="osbp", bufs=8))
        psum = ctx.enter_context(tc.tile_pool(name="psum", bufs=2,
                                              space="PSUM"))

        # ---- persistent SBUF ----
        xT_sb = const.tile([P, KD, B, T], fp)        # 64 KB/p
        wq_sb = const.tile([P, KD, CD], fp)
        wk_sb = const.tile([P, KD, CD], fp)
        wv_sb = const.tile([P, KD, CD], fp)
        wp_sb = const.tile([P, KD, CD], fp)
        bq_sb = const.tile([P, 1], f32)
        mask_sb = const.tile([P, P], fp)             # 0/1 lower triangle
        ident_sb = const.tile([P, P], fp)
        qT_sb = const.tile([P, B, T], fp)            # 2 heads stacked
        kT_sb = const.tile([P, B, T], fp)
        v_sb = const.tile([P, KT, B, HPC, DH + 1], fp)

        # weights first so the first projection matmuls start immediately;
        # x streams in per chunk behind them
        nc.sync.dma_start(wq_sb[:], wq.rearrange("(k p) c -> p k c", p=P))
        nc.sync.dma_start(wk_sb[:], wk.rearrange("(k p) c -> p k c", p=P))
        nc.sync.dma_start(wv_sb[:], wv.rearrange("(k p) c -> p k c", p=P))
        nc.sync.dma_start(wp_sb[:], wp.rearrange("(k p) c -> p k c", p=P))
        nc.sync.dma_start(bq_sb[:], bqp)
        nc.sync.dma_start(mask_sb[:], maskf)
        nc.sync.dma_start(ident_sb[:], ident)
        nc.vector.memset(v_sb[:, :, :, :, DH:DH + 1], 1.0)
        xT_r = xT.rearrange("(k p) b t -> p k b t", p=P)
        for t4 in range(NCH):
            # split the 8MB load so chunk-0 compute starts immediately
            for b in range(B):
                nc.sync.dma_start(
                    xT_sb[:, :, b, t4 * 512:(t4 + 1) * 512],
                    xT_r[:, :, b, t4 * 512:(t4 + 1) * 512])
        # tiny warmup collective: absorbs the first-collective latency
        # anomaly while the input DMAs stream
        nc.gpsimd.collective_compute(
            "AllGather", bass.mybir.AluOpType.bypass,
            replica_groups=RG, ins=[warm_in], outs=[warm_out])

        def qkv_chunk(t4):
            """Project q,k (transposed) and v (natural) for T-chunk t4."""
            for b in range(B):
                psqk = psum.tile([P, 1024], f32, tag="big",
                                 name=f"psqk_{t4}_{b}")
                for k in range(KD):
                    nc.tensor.matmul(
                        psqk[:, 0:512], wq_sb[:, k, :],
                        xT_sb[:, k, b, t4 * 512:(t4 + 1) * 512],
                        start=(k == 0), stop=(k == KD - 1))
                for k in range(KD):
                    nc.tensor.matmul(
                        psqk[:, 512:1024], wk_sb[:, k, :],
                        xT_sb[:, k, b, t4 * 512:(t4 + 1) * 512],
                        start=(k == 0), stop=(k == KD - 1))
                nc.vector.tensor_scalar_add(
                    qT_sb[:, b, t4 * 512:(t4 + 1) * 512], psqk[:, 0:512],
                    bq_sb[:, 0:1])
                nc.vector.tensor_copy(
                    kT_sb[:, b, t4 * 512:(t4 + 1) * 512], psqk[:, 512:1024])
                for tt in range(4 * t4, 4 * t4 + 4):
                    psv = psum.tile([P, 256], f32, tag="pv",
                                    name=f"psv_{tt}_{b}")
                    for k in range(KD):
                        nc.tensor.matmul(
                            psv[:, :CD], xT_sb[:, k, b, tt * P:(tt + 1) * P],
                            wv_sb[:, k, :], start=(k == 0),
                            stop=(k == KD - 1))
                    nc.vector.tensor_copy(
                        out=v_sb[:, tt, b, :, 0:DH],
                        in_=psv[:, :CD].rearrange("p (h d) -> p h d", h=HPC))

        def attention_batch(c, b):
            """Causal attention for q-chunk c, batch b (2 heads packed).

            scores for both heads go into one [128,1024] PSUM tile
            (row-packed K=64 matmuls -> halves), one fused exp per key
            tile, AV interleaved with lag 1.  AV accumulates all four
            q-subtiles of each head in one PSUM bank (4 interleaved
            accumulation groups as column ranges)."""
            nkt = 4 * (c + 1)
            exp_tiles = {}
            pso = {}
            for hh in range(2):
                pso[hh] = psum.tile([P, 4, DH + 1], f32, tag="o",
                                    name=f"pso_{c}_{b}_{hh}")

            def do_scores(k):
                ps_s = psum.tile([P, 1024], f32, tag="big",
                                 name=f"ps_{c}_{b}_{k}")
                for hh in range(2):
                    lo, hi = hh * DH, (hh + 1) * DH
                    nc.tensor.matmul(
                        ps_s[:, hh * 512:(hh + 1) * 512],
                        kT_sb[lo:hi, b, k * P:(k + 1) * P],
                        qT_sb[lo:hi, b, c * 512:(c + 1) * 512],
                        start=True, stop=True)
                e = expp.tile([P, 1024], fp, tag="expT",
                              name=f"expT_{c}_{b}_{k}")
                nc.scalar.activation(e[:], ps_s[:], AF.Exp)
                j = k - 4 * c
                if j >= 0:
                    blks = e[:].rearrange("p (hh q) -> p hh q", hh=2)[
                        :, :, j * P:(j + 1) * P]
                    nc.vector.tensor_mul(
                        blks, blks,
                        mask_sb[:, None, :].to_broadcast([P, 2, P]))
                exp_tiles[k] = e

            def do_av(k):
                # pso[hh] holds 4 interleaved accumulation groups in one
                # PSUM bank; only the first write of the bank (k==0,s==0)
                # may set start (bank-wide has_written clear).
                for hh in range(2):
                    h = 2 * b + hh  # local index only
                    for s in range(4):
                        if k <= 4 * c + s:
                            nc.tensor.matmul(
                                pso[hh][:, s, :],
                                exp_tiles[k][:, hh * 512 + s * P:
                                             hh * 512 + (s + 1) * P],
                                v_sb[:, k, b, hh, :],
                                start=(k == 0 and s == 0),
                                stop=(k == 4 * c + s),
                                skip_group_check=True)

            for k in range(nkt + 1):
                if k < nkt:
                    do_scores(k)
                if k > 0:
                    do_av(k - 1)
            return pso

        def proj_loads(c):
            """Plain DMA loads of the gathered (already transposed) heads."""
            oTk = {}
            for b2 in range(B):
                for k in range(KD):
                    t_ = otkp.tile([P, CHUNK], fp, tag="oTk",
                                   name=f"oTk_{c}_{b2}_{k}")
                    nc.sync.dma_start(t_[:], gath[c, k, b2])
                    oTk[(b2, k)] = t_
            return oTk

        def proj_chunk(c, oTk):
            """Output projection for T-chunk c (both batches in one
            2-bank PSUM tile, 4 q-subtile groups per bank)."""
            psp = psum.tile([P, B, 4, CD], f32, tag="big",
                            name=f"psp_{c}")
            for b2 in range(B):
                for k in range(KD):
                    for s in range(4):
                        nc.tensor.matmul(
                            psp[:, b2, s, :], oTk[(b2, k)][:, s * P:(s + 1) * P],
                            wp_sb[:, k, :],
                            start=(k == 0 and s == 0), stop=(k == KD - 1),
                            skip_group_check=True)
            outsb = work.tile([P, B, 4, CD], f32, tag="outsb",
                              name=f"outsb_{c}")
            nc.vector.tensor_copy(outsb[:], psp[:])
            for b2 in range(B):
                nc.sync.dma_start(
                    out[b2, c * 512:(c + 1) * 512, :].rearrange(
                        "(s p) col -> p s col", p=P),
                    outsb[:, b2])

        # pipeline: attention(c) paces ScalarE; per-chunk AllGathers fly
        # while later chunks compute; qkv(c+1) and proj(c-1) fill the PE
        # while ScalarE drains exps and AG(c) is in flight.
        oTks = {}
        qkv_chunk(0)
        for c in range(NCH):
            if c >= 2:
                oTks[c - 2] = proj_loads(c - 2)
            for b in range(B):
                pso = attention_batch(c, b)
                osb = osbp.tile([P, 4, CD], fp, tag="osb",
                                name=f"osb_{c}_{b}")
                for hh in range(2):
                    for s in range(4):
                        rec = work.tile([P, 1], f32, tag="rec",
                                        name=f"rec_{c}_{b}_{hh}_{s}")
                        nc.vector.reciprocal(rec[:],
                                             pso[hh][:, s, DH:DH + 1])
                        nc.vector.tensor_scalar_mul(
                            osb[:, s, hh * DH:(hh + 1) * DH],
                            pso[hh][:, s, 0:DH], rec[:])
                # local PE transpose: obounce carries oT so the
                # projection needs no DMA transposes
                obT = work.tile([P, 4, P], fp, tag="obT",
                                name=f"obT_{c}_{b}")
                for s in range(4):
                    trp = psum.tile([P, P], fp, tag="pv",
                                    name=f"trp_{c}_{b}_{s}")
                    nc.tensor.transpose(trp[:], osb[:, s, :], ident_sb[:])
                    nc.vector.tensor_copy(obT[:, s, :], trp[:])
                nc.sync.dma_start(
                    obounce[c, b].rearrange("p (s t) -> p s t", t=P),
                    obT[:])
            nc.gpsimd.collective_compute(
                "AllGather", bass.mybir.AluOpType.bypass,
                replica_groups=RG,
                ins=[obounce[c]],
                outs=[gath[c]])
            if c + 1 < NCH:
                qkv_chunk(c + 1)
            if c >= 2:
                # AG(c-2) completed two iterations back -> its loads and
                # matmuls never head-of-line block attention work
                proj_chunk(c - 2, oTks[c - 2])
        proj_chunk(NCH - 2, proj_loads(NCH - 2))
        proj_chunk(NCH - 1, proj_loads(NCH - 1))

    nc.finalize()
    return nc


def _get_nc():
    if "nc" not in _CACHE:
        _CACHE["nc"] = _build_nc()
    return _CACHE["nc"]


def kernel(x, Wq, bq, Wk, bk, Wv, bv, Wp, bp):
    global LAST_RESULT
    from concourse.bass_utils import run_bass_kernel_spmd

    x = np.asarray(x, dtype=np.float32)
    Wq = np.asarray(Wq, dtype=np.float32)
    Wk = np.asarray(Wk, dtype=np.float32)
    Wv = np.asarray(Wv, dtype=np.float32)
    Wp = np.asarray(Wp, dtype=np.float32)
    bq = np.asarray(bq, dtype=np.float32)
    bv = np.asarray(bv, dtype=np.float32)
    bp = np.asarray(bp, dtype=np.float32)

    s = DH ** -0.5
    maskf = np.where(
        np.arange(P)[:, None] <= np.arange(P)[None, :], 1.0, 0.0
    ).astype(np.float16)
    ident = np.eye(P, dtype=np.float16)
    xTg = np.ascontiguousarray(np.stack([x[0].T, x[1].T], axis=1)
                               ).astype(np.float16)

    in_maps = []
    for r in range(NCORES):
        cols = slice(r * CD, (r + 1) * CD)
        in_maps.append({
            "xT": xTg,
            "wq": (Wq[:, cols] * s).astype(np.float16),
            "wk": np.ascontiguousarray(Wk[:, cols]).astype(np.float16),
            "wv": np.ascontiguousarray(Wv[:, cols]).astype(np.float16),
            "wp": np.ascontiguousarray(Wp[:, cols]).astype(np.float16),
            "bqp": np.ascontiguousarray((bq[cols] * s).reshape(P, 1)),
            "maskf": maskf,
            "ident": ident,
        })

    nc = _get_nc()
    res = run_bass_kernel_spmd(
        nc, in_maps, core_ids=list(range(NCORES)),
        trace=bool(int(os.environ.get("KERNEL_TRACE", "0"))))
    LAST_RESULT = res

    out = np.empty((B, T, D), dtype=np.float32)
    for r in range(NCORES):
        out[:, :, r * CD:(r + 1) * CD] = res.results[r]["out"]
    # bias terms that are constant w.r.t. the data path:
    #   v-bias passes through softmax rows (sum=1) -> + bv@Wp; plus bp.
    #   (bk shifts every logit in a row equally -> cancels in softmax.)
    out += (bv @ Wp + bp)[None, None, :]
    return out


# revision 43
# speedup vs baseline: 1.4625x; 1.0213x over previous
"""Distributed causal multi-head attention for TRN2 (8 NeuronCores).

Problem: B=2, T=2048, D=1024, H=16 heads (head_dim 64), causal MHA:
  q,k,v = x@W{q,k,v}+b, q *= dh**-0.5, o = softmax(mask(q k^T)) v, out = o@Wp + bp

Sharding: 8-way tensor parallel over heads.  Core r handles BOTH batches,
heads {2r, 2r+1}, and output columns 128r..128(r+1).  This makes the
AllGather a single uniform 8-core collective (the 4-core-group path
measured ~50us/call vs ~7us for 8-core) with every gathered byte used by
every core.  Per core:
  - QKV projections in fp16 on TensorE (q/k produced transposed [hd, t],
    v produced natural [t, hd] with an appended ones-column)
  - scores computed transposed [keys, q] (K=64 contraction, two heads
    packed into the 128x128 PE array via row tiling, both writing halves
    of one 2-bank PSUM tile); ONE fused exp per key tile on ScalarE
    (the pipeline pacer); causal handled by key-tile skipping + a
    post-exp 0/1 mask multiply on the diagonal blocks
  - AV uses exp-weights as the stationary operand -> o natural [q, hd]
    with per-partition row sums for free (ones column of v); normalize
    with a per-partition reciprocal; AV interleaves with scores at lag 1
    so the PE never waits long on ScalarE
  - o is PE-transposed locally (cheap) so the AllGather carries oT and
    the output projection needs no DMA transposes
  - output projection computes a 128-column slice per core, pipelined
    one chunk behind the AllGather.
Host side only shards/converts inputs, concatenates outputs, and adds
the bias terms that are mathematically output-constant (bv@Wp + bp; bk
cancels in softmax; bq is applied on device).
"""

import os
import numpy as np

B, T, D, H = 2, 2048, 1024, 16
DH = 64
NCORES = 8
HPC = H // NCORES      # heads per core = 2
CD = HPC * DH          # per-core head-dim / out columns = 128
P = 128
NCH = 4                # T chunks for the AllGather pipeline
CHUNK = T // NCH       # 512
KT = T // P            # 16 key tiles
KD = D // P            # 8 contraction tiles for the projections

_CACHE = {}

# Results of the last device run (for test harnesses): BassKernelResults
LAST_RESULT = None


def _build_nc():
    import concourse.bass as bass
    import concourse.mybir as mybir
    import concourse.tile as tile
    from concourse import bacc
    from contextlib import ExitStack

    fp = mybir.dt.float16
    f32 = mybir.dt.float32
    AF = mybir.ActivationFunctionType

    nc = bacc.Bacc("TRN2", target_bir_lowering=False, debug=False,
                   num_devices=NCORES)

    xT = nc.dram_tensor("xT", [D, B, T], fp, kind="ExternalInput").ap()
    wq = nc.dram_tensor("wq", [D, CD], fp, kind="ExternalInput").ap()
    wk = nc.dram_tensor("wk", [D, CD], fp, kind="ExternalInput").ap()
    wv = nc.dram_tensor("wv", [D, CD], fp, kind="ExternalInput").ap()
    wp = nc.dram_tensor("wp", [D, CD], fp, kind="ExternalInput").ap()
    bqp = nc.dram_tensor("bqp", [P, 1], f32, kind="ExternalInput").ap()
    maskf = nc.dram_tensor("maskf", [P, P], fp, kind="ExternalInput").ap()
    ident = nc.dram_tensor("ident", [P, P], fp, kind="ExternalInput").ap()
    out = nc.dram_tensor("out", [B, T, CD], f32, kind="ExternalOutput").ap()

    obounce = nc.dram_tensor("obounce", [NCH, B, CD, CHUNK], fp).ap()
    gath = nc.dram_tensor("gath", [NCH, NCORES, B, CD, CHUNK], fp).ap()
    # chunk 3 gathers per batch (smaller, earlier) to shrink the tail
    gath3 = nc.dram_tensor("gath3", [B, NCORES, CD, CHUNK], fp).ap()
    warm_in = nc.dram_tensor("warm_in", [P], fp).ap()
    warm_out = nc.dram_tensor("warm_out", [NCORES, P], fp).ap()

    RG = [[0, 1, 2, 3, 4, 5, 6, 7]]

    with tile.TileContext(nc, num_cores=NCORES) as tc, ExitStack() as ctx:
        const = ctx.enter_context(tc.tile_pool(name="const", bufs=1))
        work = ctx.enter_context(tc.tile_pool(name="work", bufs=3))
        expp = ctx.enter_context(tc.tile_pool(name="expp", bufs=18))
        otkp = ctx.enter_context(tc.tile_pool(name="otkp", bufs=18))
        osbp = ctx.enter_context(tc.tile_pool(name="osbp", bufs=8))
        psum = ctx.enter_context(tc.tile_pool(name="psum", bufs=2,
                                              space="PSUM"))

        # ---- persistent SBUF ----
        xT_sb = const.tile([P, KD, B, T], fp)        # 64 KB/p
        wq_sb = const.tile([P, KD, CD], fp)
        wk_sb = const.tile([P, KD, CD], fp)
        wv_sb = const.tile([P, KD, CD], fp)
        wp_sb = const.tile([P, KD, CD], fp)
        bq_sb = const.tile([P, 1], f32)
        mask_sb = const.tile([P, P], fp)             # 0/1 lower triangle
        ident_sb = const.tile([P, P], fp)
        qT_sb = const.tile([P, B, T], fp)            # 2 heads stacked
        kT_sb = const.tile([P, B, T], fp)
        v_sb = const.tile([P, KT, B, HPC, DH + 1], fp)

        # weights first so the first projection matmuls start immediately;
        # x streams in per chunk behind them
        nc.sync.dma_start(wq_sb[:], wq.rearrange("(k p) c -> p k c", p=P))
        nc.sync.dma_start(wk_sb[:], wk.rearrange("(k p) c -> p k c", p=P))
        nc.sync.dma_start(wv_sb[:], wv.rearrange("(k p) c -> p k c", p=P))
        nc.sync.dma_start(wp_sb[:], wp.rearrange("(k p) c -> p k c", p=P))
        nc.sync.dma_start(bq_sb[:], bqp)
        nc.sync.dma_start(mask_sb[:], maskf)
        nc.sync.dma_start(ident_sb[:], ident)
        nc.vector.memset(v_sb[:, :, :, :, DH:DH + 1], 1.0)
        xT_r = xT.rearrange("(k p) b t -> p k b t", p=P)
        for t4 in range(NCH):
            # split the 8MB load so chunk-0 compute starts immediately
            for b in range(B):
                nc.sync.dma_start(
                    xT_sb[:, :, b, t4 * 512:(t4 + 1) * 512],
                    xT_r[:, :, b, t4 * 512:(t4 + 1) * 512])
        # tiny warmup collective: absorbs the first-collective latency
        # anomaly while the input DMAs stream
        nc.gpsimd.collective_compute(
            "AllGather", bass.mybir.AluOpType.bypass,
            replica_groups=RG, ins=[warm_in], outs=[warm_out])

        def qkv_units(t4):
            """Projection work for T-chunk t4 as a list of closures, so
            it can be drip-fed into the attention k-loop (fills the PE
            while ScalarE paces the exp pipeline)."""
            units = []

            def qk_unit(b):
                psqk = psum.tile([P, 1024], f32, tag="big", bufs=3,
                                 name=f"psqk_{t4}_{b}")
                for k in range(KD):
                    nc.tensor.matmul(
                        psqk[:, 0:512], wq_sb[:, k, :],
                        xT_sb[:, k, b, t4 * 512:(t4 + 1) * 512],
                        start=(k == 0), stop=(k == KD - 1))
                for k in range(KD):
                    nc.tensor.matmul(
                        psqk[:, 512:1024], wk_sb[:, k, :],
                        xT_sb[:, k, b, t4 * 512:(t4 + 1) * 512],
                        start=(k == 0), stop=(k == KD - 1))
                nc.vector.tensor_scalar_add(
                    qT_sb[:, b, t4 * 512:(t4 + 1) * 512], psqk[:, 0:512],
                    bq_sb[:, 0:1])
                nc.vector.tensor_copy(
                    kT_sb[:, b, t4 * 512:(t4 + 1) * 512], psqk[:, 512:1024])

            def v_unit(b, tt):
                psv = psum.tile([P, 256], f32, tag="big", bufs=3,
                                name=f"psv_{tt}_{b}")
                for k in range(KD):
                    nc.tensor.matmul(
                        psv[:, :CD], xT_sb[:, k, b, tt * P:(tt + 1) * P],
                        wv_sb[:, k, :], start=(k == 0),
                        stop=(k == KD - 1))
                nc.vector.tensor_copy(
                    out=v_sb[:, tt, b, :, 0:DH],
                    in_=psv[:, :CD].rearrange("p (h d) -> p h d", h=HPC))

            for b in range(B):
                units.append(lambda b=b: qk_unit(b))
                for tt in range(4 * t4, 4 * t4 + 4):
                    units.append(lambda b=b, tt=tt: v_unit(b, tt))
            return units

        def qkv_chunk(t4):
            for u in qkv_units(t4):
                u()

        def attention_batch(c, b, filler=None):
            """Causal attention for q-chunk c, batch b (2 heads packed).

            scores for both heads go into one [128,1024] PSUM tile
            (row-packed K=64 matmuls -> halves), one fused exp per key
            tile, AV interleaved with lag 1.  AV accumulates all four
            q-subtiles of each head in one PSUM bank (4 interleaved
            accumulation groups as column ranges)."""
            nkt = 4 * (c + 1)
            exp_tiles = {}
            pso = {}
            for hh in range(2):
                pso[hh] = psum.tile([P, 4, DH + 1], f32, tag="o",
                                    name=f"pso_{c}_{b}_{hh}")

            def do_scores(k):
                ps_s = psum.tile([P, 1024], f32, tag="big", bufs=3,
                                 name=f"ps_{c}_{b}_{k}")
                for hh in range(2):
                    lo, hi = hh * DH, (hh + 1) * DH
                    nc.tensor.matmul(
                        ps_s[:, hh * 512:(hh + 1) * 512],
                        kT_sb[lo:hi, b, k * P:(k + 1) * P],
                        qT_sb[lo:hi, b, c * 512:(c + 1) * 512],
                        start=True, stop=True)
                e = expp.tile([P, 1024], fp, tag="expT",
                              name=f"expT_{c}_{b}_{k}")
                nc.scalar.activation(e[:], ps_s[:], AF.Exp)
                j = k - 4 * c
                if j >= 0:
                    blks = e[:].rearrange("p (hh q) -> p hh q", hh=2)[
                        :, :, j * P:(j + 1) * P]
                    nc.vector.tensor_mul(
                        blks, blks,
                        mask_sb[:, None, :].to_broadcast([P, 2, P]))
                exp_tiles[k] = e

            def do_av(k):
                # pso[hh] holds 4 interleaved accumulation groups in one
                # PSUM bank; only the first write of the bank (k==0,s==0)
                # may set start (bank-wide has_written clear).
                for hh in range(2):
                    h = 2 * b + hh  # local index only
                    for s in range(4):
                        if k <= 4 * c + s:
                            nc.tensor.matmul(
                                pso[hh][:, s, :],
                                exp_tiles[k][:, hh * 512 + s * P:
                                             hh * 512 + (s + 1) * P],
                                v_sb[:, k, b, hh, :],
                                start=(k == 0 and s == 0),
                                stop=(k == 4 * c + s),
                                skip_group_check=True)

            for k in range(nkt + 1):
                if k < nkt:
                    do_scores(k)
                if k > 0:
                    do_av(k - 1)
                if filler is not None and k >= 2:
                    filler()
            return pso

        def proj_loads(c):
            """Plain DMA loads of the gathered (already transposed) heads."""
            oTk = {}
            for b2 in range(B):
                for k in range(KD):
                    t_ = otkp.tile([P, CHUNK], fp, tag="oTk",
                                   name=f"oTk_{c}_{b2}_{k}")
                    nc.sync.dma_start(t_[:], gath[c, k, b2])
                    oTk[(b2, k)] = t_
            return oTk

        def proj_batch(c, b2, oTk):
            """Output projection for T-chunk c, one batch (one PSUM bank
            holding 4 interleaved q-subtile accumulation groups)."""
            psp = psum.tile([P, 4, CD], f32, tag="o",
                            name=f"psp_{c}_{b2}")
            for k in range(KD):
                for s in range(4):
                    nc.tensor.matmul(
                        psp[:, s, :], oTk[(b2, k)][:, s * P:(s + 1) * P],
                        wp_sb[:, k, :],
                        start=(k == 0 and s == 0), stop=(k == KD - 1),
                        skip_group_check=True)
            outsb = work.tile([P, 4, CD], f32, tag="outsb",
                              name=f"outsb_{c}_{b2}")
            nc.vector.tensor_copy(outsb[:], psp[:])
            nc.sync.dma_start(
                out[b2, c * 512:(c + 1) * 512, :].rearrange(
                    "(s p) col -> p s col", p=P),
                outsb[:])

        def proj_chunk(c, oTk):
            for b2 in range(B):
                proj_batch(c, b2, oTk)

        def finish_batch(c, b, pso):
            """normalize -> PE transpose -> bounce buffer for AG."""
            osb = osbp.tile([P, 4, CD], fp, tag="osb",
                            name=f"osb_{c}_{b}")
            for hh in range(2):
                for s in range(4):
                    rec = work.tile([P, 1], f32, tag="rec",
                                    name=f"rec_{c}_{b}_{hh}_{s}")
                    nc.vector.reciprocal(rec[:],
                                         pso[hh][:, s, DH:DH + 1])
                    nc.vector.tensor_scalar_mul(
                        osb[:, s, hh * DH:(hh + 1) * DH],
                        pso[hh][:, s, 0:DH], rec[:])
            # local PE transpose: obounce carries oT so the projection
            # needs no DMA transposes
            obT = work.tile([P, 4, P], fp, tag="obT", name=f"obT_{c}_{b}")
            for s in range(4):
                trp = psum.tile([P, P], fp, tag="o",
                                name=f"trp_{c}_{b}_{s}")
                nc.tensor.transpose(trp[:], osb[:, s, :], ident_sb[:])
                nc.vector.tensor_copy(obT[:, s, :], trp[:])
            nc.sync.dma_start(
                obounce[c, b].rearrange("p (s t) -> p s t", t=P), obT[:])

        # pipeline: attention(c) paces ScalarE; per-chunk AllGathers fly
        # while later chunks compute; qkv(c+1) is drip-fed INTO the
        # attention k-loop so the PE fills ScalarE-paced slack instead of
        # idling ScalarE afterwards; proj(c-2) runs behind a completed AG.
        # The last chunk gathers and projects per batch to shrink the tail.
        oTks = {}
        pending = list(qkv_units(0))
        for u in pending:
            u()
        pending = []

        def filler():
            if pending:
                pending.pop(0)()

        for c in range(NCH):
            if c >= 2:
                oTks[c - 2] = proj_loads(c - 2)
            pending.extend(qkv_units(c + 1) if c + 1 < NCH else [])
            for b in range(B):
                pso = attention_batch(c, b, filler=filler)
                finish_batch(c, b, pso)
                if c == NCH - 1:
                    nc.gpsimd.collective_compute(
                        "AllGather", bass.mybir.AluOpType.bypass,
                        replica_groups=RG,
                        ins=[obounce[c, b]],
                        outs=[gath3[b]])
            if c < NCH - 1:
                nc.gpsimd.collective_compute(
                    "AllGather", bass.mybir.AluOpType.bypass,
                    replica_groups=RG,
                    ins=[obounce[c]],
                    outs=[gath[c]])
            while pending:
                pending.pop(0)()
            if c >= 2:
                # AG(c-2) completed two iterations back -> its loads and
                # matmuls never head-of-line block attention work
                proj_chunk(c - 2, oTks[c - 2])
        proj_chunk(NCH - 2, proj_loads(NCH - 2))
        for b2 in range(B):
            oTk3 = {}
            for k in range(KD):
                t_ = otkp.tile([P, CHUNK], fp, tag="oTk",
                               name=f"oTk3_{b2}_{k}")
                nc.sync.dma_start(t_[:], gath3[b2, k])
                oTk3[(b2, k)] = t_
            proj_batch(NCH - 1, b2, oTk3)

    nc.finalize()
    return nc


def _get_nc():
    if "nc" not in _CACHE:
        _CACHE["nc"] = _build_nc()
    return _CACHE["nc"]


def kernel(x, Wq, bq, Wk, bk, Wv, bv, Wp, bp):
    global LAST_RESULT
    from concourse.bass_utils import run_bass_kernel_spmd

    x = np.asarray(x, dtype=np.float32)
    Wq = np.asarray(Wq, dtype=np.float32)
    Wk = np.asarray(Wk, dtype=np.float32)
    Wv = np.asarray(Wv, dtype=np.float32)
    Wp = np.asarray(Wp, dtype=np.float32)
    bq = np.asarray(bq, dtype=np.float32)
    bv = np.asarray(bv, dtype=np.float32)
    bp = np.asarray(bp, dtype=np.float32)

    s = DH ** -0.5
    maskf = np.where(
        np.arange(P)[:, None] <= np.arange(P)[None, :], 1.0, 0.0
    ).astype(np.float16)
    ident = np.eye(P, dtype=np.float16)
    xTg = np.ascontiguousarray(np.stack([x[0].T, x[1].T], axis=1)
                               ).astype(np.float16)

    in_maps = []
    for r in range(NCORES):
        cols = slice(r * CD, (r + 1) * CD)
        in_maps.append({
            "xT": xTg,
            "wq": (Wq[:, cols] * s).astype(np.float16),
            "wk": np.ascontiguousarray(Wk[:, cols]).astype(np.float16),
            "wv": np.ascontiguousarray(Wv[:, cols]).astype(np.float16),
            "wp": np.ascontiguousarray(Wp[:, cols]).astype(np.float16),
            "bqp": np.ascontiguousarray((bq[cols] * s).reshape(P, 1)),
            "maskf": maskf,
            "ident": ident,
        })

    nc = _get_nc()
    res = run_bass_kernel_spmd(
        nc, in_maps, core_ids=list(range(NCORES)),
        trace=bool(int(os.environ.get("KERNEL_TRACE", "0"))))
    LAST_RESULT = res

    out = np.empty((B, T, D), dtype=np.float32)
    for r in range(NCORES):
        out[:, :, r * CD:(r + 1) * CD] = res.results[r]["out"]
    # bias terms that are constant w.r.t. the data path:
    #   v-bias passes through softmax rows (sum=1) -> + bv@Wp; plus bp.
    #   (bk shifts every logit in a row equally -> cancels in softmax.)
    out += (bv @ Wp + bp)[None, None, :]
    return out
